# revision 1
# baseline (speedup 1.0000x reference)
"""PointContrastive loss on 8 Trainium2 NeuronCores.

Sharding: data-parallel over objects (2 objects/core). Each core computes its
64 masks' avg features via fp32r segment-matmuls, all-gathers the transposed
avg feats, then computes its row/col slices of the 512x512 contrastive logits
and the per-mask log-softmax diagonal losses. The host combines the final
nonzero-means (1 KB of data).

Device layouts are chosen so every DMA is a plain contiguous copy:
 - maskT:   [128, 64*64]  block-diagonal transposed masks (chunk-major)
 - pts:     [8192, 768]   this core's net_out slab
 - embsT:   [768, 512]    mask_embs transposed (all cores identical)
 - embs_locT: [768, 64]   this core's column slice of embsT
 - embs_loc:  [64, 768]   this core's row slice of mask_embs
"""

import numpy as np
import concourse.bass as bass
from concourse import bacc
import concourse.mybir as mybir
import concourse.tile as tile
from concourse.bass_utils import run_bass_kernel_spmd
from concourse.masks import make_identity

BS, P, M, D = 16, 4096, 32, 768
NCORES = 8
OBJ = BS // NCORES          # objects per core = 2
NM = OBJ * M                # local masks = 64
N = BS * M                  # global masks = 512
KPO = P // 128              # k-chunks per object = 32
NCH = OBJ * KPO             # k-chunks per core = 64
DCH = D // 128              # feature chunks = 6

f32 = mybir.dt.float32
f32r = mybir.dt.float32r

_nc_cache = None


def _build():
    nc = bacc.Bacc("TRN2", target_bir_lowering=False, debug=False, num_devices=NCORES)
    maskT_d = nc.dram_tensor("maskT", [128, NCH * NM], f32r, kind="ExternalInput").ap()
    pts_d = nc.dram_tensor("pts", [NCH * 128, D], f32r, kind="ExternalInput").ap()
    embsT_d = nc.dram_tensor("embsT", [D, N], f32r, kind="ExternalInput").ap()
    elocT_d = nc.dram_tensor("elocT", [D, NM], f32r, kind="ExternalInput").ap()
    eloc_d = nc.dram_tensor("eloc", [NM, D], f32, kind="ExternalInput").ap()
    ones_d = nc.dram_tensor("ones", [128, 2], f32r, kind="ExternalInput").ap()
    sca_d = nc.dram_tensor("sca", [NM, 1], f32, kind="ExternalInput").ap()
    out_d = nc.dram_tensor("out", [NM, 4], f32, kind="ExternalOutput").ap()

    with tile.TileContext(nc) as tc:
        with (
            tc.tile_pool(name="sb", bufs=1) as sb,
            tc.tile_pool(name="pts_pool", bufs=6) as pts_pool,
            tc.tile_pool(name="ps", bufs=1, space="PSUM") as ps,
            tc.tile_pool(name="pst", bufs=2, space="PSUM") as pst,
            tc.tile_pool(name="dram", bufs=1, space="DRAM") as dram,
        ):
            # ---- persistent small loads ----
            ones = sb.tile([128, 2], f32r)
            nc.sync.dma_start(ones[:], ones_d[:])
            sca = sb.tile([NM, 1], f32)
            nc.sync.dma_start(sca[:], sca_d[:])
            ident = sb.tile([128, 128], f32)
            make_identity(nc, ident[:])
            mt = sb.tile([128, NCH * NM], f32r)
            nc.sync.dma_start(mt[:], maskT_d[:])
            el = sb.tile([NM, D], f32)
            nc.sync.dma_start(el[:], eloc_d[:])
            et = []
            elt = []
            for j in range(DCH):
                e1 = sb.tile([128, N], f32r, name=f"et{j}")
                nc.sync.dma_start(e1[:], embsT_d[j * 128:(j + 1) * 128, :])
                et.append(e1)
                e2 = sb.tile([128, NM], f32r, name=f"elt{j}")
                nc.sync.dma_start(e2[:], elocT_d[j * 128:(j + 1) * 128, :])
                elt.append(e2)

            # ---- phase 1: segment sums over 64 k-chunks ----
            s_ps = ps.tile([NM, D], f32)
            n_ps = ps.tile([NM, 2], f32)
            for c in range(NCH):
                pt = pts_pool.tile([128, D], f32r, name="pt")
                nc.sync.dma_start(pt[:], pts_d[c * 128:(c + 1) * 128, :])
                lhs = mt[:, c * NM:(c + 1) * NM]
                st, sp = (c == 0), (c == NCH - 1)
                nc.tensor.matmul(s_ps[:, 0:512], lhs, pt[:, 0:512], start=st, stop=sp)
                nc.tensor.matmul(s_ps[:, 512:D], lhs, pt[:, 512:D], start=st, stop=sp)
                nc.tensor.matmul(n_ps[:], lhs, ones[:], start=st, stop=sp)

            npts = sb.tile([NM, 1], f32)
            nc.vector.tensor_copy(npts[:], n_ps[:, 0:1])
            t1 = sb.tile([NM, 1], f32)
            nc.vector.tensor_scalar_add(t1[:], npts[:], 1e-12)
            inv = sb.tile([NM, 1], f32)
            nc.vector.reciprocal(inv[:], t1[:])
            a = sb.tile([NM, D], f32)
            nc.scalar.activation(a[:], s_ps[:], mybir.ActivationFunctionType.Copy,
                                 bias=0.0, scale=inv[:])

            # diag[i] = eloc[i] . a[i]
            dtmp = sb.tile([NM, D], f32)
            nc.vector.tensor_mul(dtmp[:], a[:], el[:])
            diag = sb.tile([NM, 1], f32)
            nc.vector.reduce_sum(diag[:], dtmp[:], axis=mybir.AxisListType.X)

            # ---- transpose A locally, all-gather A^T ----
            at = []
            for j in range(DCH):
                tp = pst.tile([128, NM], f32, name="tp")
                nc.tensor.transpose(tp[:], a[:, j * 128:(j + 1) * 128],
                                    ident[0:NM, 0:NM])
                a1 = sb.tile([128, NM], f32r, name=f"at{j}")
                nc.vector.tensor_copy(a1[:], tp[:])
                at.append(a1)

            cc_in = dram.tile([D, NM], f32r)
            cc_out = dram.tile([NCORES * D, NM], f32r)
            for j in range(DCH):
                nc.sync.dma_start(cc_in[j * 128:(j + 1) * 128, :], at[j][:])
            nc.gpsimd.collective_compute(
                "AllGather",
                mybir.AluOpType.bypass,
                replica_groups=[list(range(NCORES))],
                ins=[cc_in[:].opt()],
                outs=[cc_out[:].opt()],
            )
            aat = []
            for j in range(DCH):
                a2 = sb.tile([128, N], f32r, name=f"aat{j}")
                for r in range(NCORES):
                    nc.sync.dma_start(
                        a2[:, r * NM:(r + 1) * NM],
                        cc_out[r * D + j * 128:r * D + (j + 1) * 128, :],
                    )
                aat.append(a2)

            # ---- phase 2: logit slices ----
            r_ps = ps.tile([NM, N], f32)
            c_ps = ps.tile([NM, N], f32)
            for j in range(DCH):
                st, sp = (j == 0), (j == DCH - 1)
                nc.tensor.matmul(r_ps[:], elt[j][:], aat[j][:], start=st, stop=sp)
                nc.tensor.matmul(c_ps[:], at[j][:], et[j][:], start=st, stop=sp)

            out_sb = sb.tile([NM, 4], f32)

            def lse_loss(logits_ps, col, scratch_name):
                # loss = log(sum_j exp(s*l_j - s*m)) + s*m - s*diag
                m = sb.tile([NM, 1], f32, name=f"m_{scratch_name}")
                nc.vector.reduce_max(m[:], logits_ps[:], axis=mybir.AxisListType.X)
                negsm = sb.tile([NM, 1], f32, name=f"negsm_{scratch_name}")
                nc.vector.tensor_mul(negsm[:], m[:], sca[:])
                nc.vector.tensor_scalar_mul(negsm[:], negsm[:], -1.0)
                ex = sb.tile([NM, N], f32, name=f"ex_{scratch_name}")
                se = sb.tile([NM, 1], f32, name=f"se_{scratch_name}")
                nc.scalar.activation(ex[:], logits_ps[:],
                                     mybir.ActivationFunctionType.Exp,
                                     bias=negsm[:], scale=sca[:], accum_out=se[:])
                lse = sb.tile([NM, 1], f32, name=f"lse_{scratch_name}")
                nc.scalar.activation(lse[:], se[:], mybir.ActivationFunctionType.Ln)
                t = sb.tile([NM, 1], f32, name=f"t_{scratch_name}")
                nc.vector.tensor_sub(t[:], m[:], diag[:])
                nc.vector.tensor_mul(t[:], t[:], sca[:])
                nc.vector.tensor_add(out_sb[:, col:col + 1], lse[:], t[:])

            lse_loss(r_ps, 0, "r")   # texts_loss: row-wise over R
            lse_loss(c_ps, 1, "c")   # pts_loss: row-wise over C^T
            nc.vector.tensor_copy(out_sb[:, 2:3], npts[:])
            nc.vector.tensor_copy(out_sb[:, 3:4], diag[:])
            nc.sync.dma_start(out_d[:], out_sb[:])
    nc.compile()
    return nc


def _prep_inputs(net_out, mask_embs, mask_pts, logit_scale):
    net_out = np.ascontiguousarray(np.asarray(net_out, dtype=np.float32))
    mask_embs = np.ascontiguousarray(np.asarray(mask_embs, dtype=np.float32))
    mask_pts = np.ascontiguousarray(np.asarray(mask_pts, dtype=np.float32))
    s = float(np.exp(np.float64(np.asarray(logit_scale).reshape(-1)[0])))

    # block-diagonal transposed masks: h[c, p, b*KPO+k, b*M+m] = mask[2c+b, m, k*128+p]
    v = mask_pts.reshape(NCORES, OBJ, M, KPO, 128).transpose(0, 1, 4, 3, 2)
    h = np.zeros((NCORES, 128, NCH, NM), dtype=np.float32)
    for b in range(OBJ):
        h[:, :, b * KPO:(b + 1) * KPO, b * M:(b + 1) * M] = v[:, b]
    embsT = np.ascontiguousarray(mask_embs.T)
    ones = np.ones((128, 2), dtype=np.float32)
    sca = np.full((NM, 1), s, dtype=np.float32)

    in_maps = []
    for c in range(NCORES):
        in_maps.append({
            "maskT": np.ascontiguousarray(h[c].reshape(128, NCH * NM)),
            "pts": np.ascontiguousarray(
                net_out[c * OBJ * P:(c + 1) * OBJ * P, :]),
            "embsT": embsT,
            "elocT": np.ascontiguousarray(embsT[:, c * NM:(c + 1) * NM]),
            "eloc": np.ascontiguousarray(mask_embs[c * NM:(c + 1) * NM, :]),
            "ones": ones,
            "sca": sca,
        })
    return in_maps


def _nonzero_mean(x):
    nz = x > 0
    cnt = int(nz.sum())
    if cnt == 0:
        return np.float32(0.0)
    return np.where(nz, x, 0.0).sum(dtype=np.float64) / cnt


def _combine(results):
    outs = [np.asarray(results[c]["out"]) for c in range(NCORES)]
    texts = np.concatenate([o[:, 0] for o in outs])
    ptsl = np.concatenate([o[:, 1] for o in outs])
    npts = np.concatenate([o[:, 2] for o in outs])
    valid = npts > 0
    texts = np.where(valid, texts, 0.0)
    ptsl = np.where(valid, ptsl, 0.0)
    return np.asarray(
        (_nonzero_mean(texts) + _nonzero_mean(ptsl)) / 2.0, dtype=np.float32)


def _run(trace=False, **inputs):
    global _nc_cache
    if _nc_cache is None:
        _nc_cache = _build()
    in_maps = _prep_inputs(
        inputs["net_out"], inputs["mask_embs"], inputs["mask_pts"],
        inputs["logit_scale"])
    res = run_bass_kernel_spmd(
        _nc_cache, in_maps, core_ids=list(range(NCORES)), trace=trace)
    return _combine(res.results), res


def kernel(**inputs) -> np.ndarray:
    out, _ = _run(trace=False, **inputs)
    return out


# revision 8
# speedup vs baseline: 1.3973x; 1.3973x over previous
"""PointContrastive loss on 8 Trainium2 NeuronCores.

Sharding: data-parallel over objects (2 objects/core). Each core computes its
64 masks' avg features via bf16 segment-matmuls (mask values are exactly
representable; fp32 PSUM accumulate), then computes BOTH orientations of its
local logit slab from purely local data:
  C^T = A_loc @ E_all^T   [64 local j, 512 global i]  -> pts_loss locally
  G   = E_all @ A_loc^T   [512 global i, 64 local j]  -> texts-direction
        row partials (max / sumexp over the local 64 columns)
Only the [512,2] row-stat partials are all-gathered (4 KB/rank instead of the
196 KB avg-feature gather), so phase 2 has no collective on its critical path.
The host finishes with the two nonzero-means (1 KB of data).

Device input layouts are chosen so every DMA is a contiguous copy:
 - maskT: [128, 64*64] bf16  block-diagonal transposed masks (chunk-major)
 - pts:   [8192, 770] bf16   net_out slab + two ones-columns (npts comes out
          of the same accumulating matmul, column 768)
 - embsT: [768, 512] f32r    mask_embs transposed (same on all cores)
 - eloc:  [64, 768] f32      this core's row slice of mask_embs
"""

import numpy as np
import ml_dtypes
import concourse.bass as bass
from concourse import bacc
import concourse.mybir as mybir
import concourse.tile as tile
from concourse.bass_utils import run_bass_kernel_spmd
from concourse.masks import make_identity

BS, P, M, D = 16, 4096, 32, 768
NCORES = 8
OBJ = BS // NCORES          # objects per core = 2
NM = OBJ * M                # local masks = 64
N = BS * M                  # global masks = 512
KPO = P // 128              # k-chunks per object = 32
NCH = OBJ * KPO             # k-chunks per core = 64
DCH = D // 128              # feature chunks = 6
DP = D + 2                  # pts row width incl. ones columns = 770
ITS = N // 128              # row tiles of the global mask axis = 4

f32 = mybir.dt.float32
f32r = mybir.dt.float32r
bf16 = mybir.dt.bfloat16

_nc_cache = None


def _build():
    nc = bacc.Bacc("TRN2", target_bir_lowering=False, debug=False, num_devices=NCORES)
    maskT_d = nc.dram_tensor("maskT", [128, NCH * NM], bf16, kind="ExternalInput").ap()
    pts_d = nc.dram_tensor("pts", [NCH * 128, DP], bf16, kind="ExternalInput").ap()
    embsT_d = nc.dram_tensor("embsT", [D, N], f32r, kind="ExternalInput").ap()
    eloc_d = nc.dram_tensor("eloc", [NM, D], f32, kind="ExternalInput").ap()
    sca_d = nc.dram_tensor("sca", [128, 1], f32, kind="ExternalInput").ap()
    out_d = nc.dram_tensor("out", [NM, 4], f32, kind="ExternalOutput").ap()
    lse_d = nc.dram_tensor("lse", [128, ITS], f32, kind="ExternalOutput").ap()
    dbg_d = nc.dram_tensor("dbg", [128, 2 * ITS + 2 * ITS * NCORES], f32, kind="ExternalOutput").ap()

    with tile.TileContext(nc) as tc:
        with (
            tc.tile_pool(name="sb", bufs=1) as sb,
            tc.tile_pool(name="pts_pool", bufs=4) as pts_pool,
            tc.tile_pool(name="ps", bufs=1, space="PSUM") as ps,
            tc.tile_pool(name="pst", bufs=2, space="PSUM") as pst,
            tc.tile_pool(name="dram", bufs=1, space="DRAM") as dram,
        ):
            # ---- persistent small loads ----
            sca = sb.tile([128, 1], f32)
            nc.sync.dma_start(sca[:], sca_d[:])
            ident = sb.tile([128, 128], f32)
            make_identity(nc, ident[:])
            mt = sb.tile([128, NCH * NM], bf16)
            nc.sync.dma_start(mt[:], maskT_d[:])
            el = sb.tile([NM, D], f32)
            nc.sync.dma_start(el[:], eloc_d[:])
            et = []
            for j in range(DCH):
                e1 = sb.tile([128, N], f32r, name=f"et{j}")
                nc.sync.dma_start(e1[:], embsT_d[j * 128:(j + 1) * 128, :])
                et.append(e1)

            # ---- phase 1: segment sums over 64 k-chunks (2 chunks per DMA) ----
            s_ps = ps.tile([NM, DP], f32)
            for t in range(NCH // 2):
                pt = pts_pool.tile([128, 2, DP], bf16, name="pt")
                nc.sync.dma_start(
                    pt[:],
                    pts_d[t * 256:(t + 1) * 256, :].rearrange("(c p) m -> p c m", p=128),
                )
                for q in range(2):
                    c = 2 * t + q
                    lhs = mt[:, c * NM:(c + 1) * NM]
                    st, sp = (c == 0), (c == NCH - 1)
                    nc.tensor.matmul(s_ps[:, 0:512], lhs, pt[:, q, 0:512],
                                     start=st, stop=sp)
                    nc.tensor.matmul(s_ps[:, 512:DP], lhs, pt[:, q, 512:DP],
                                     start=st, stop=sp)

            npts = sb.tile([NM, 1], f32)
            nc.vector.tensor_copy(npts[:], s_ps[:, D:D + 1])
            t1 = sb.tile([NM, 1], f32)
            nc.vector.tensor_scalar_add(t1[:], npts[:], 1e-12)
            inv = sb.tile([NM, 1], f32)
            nc.vector.reciprocal(inv[:], t1[:])
            a = sb.tile([NM, D], f32)
            nc.scalar.activation(a[:], s_ps[:, 0:D], mybir.ActivationFunctionType.Copy,
                                 bias=0.0, scale=inv[:])

            # diag[i] = eloc[i] . a[i]
            dtmp = sb.tile([NM, D], f32)
            nc.vector.tensor_mul(dtmp[:], a[:], el[:])
            diag = sb.tile([NM, 1], f32)
            nc.vector.reduce_sum(diag[:], dtmp[:], axis=mybir.AxisListType.X)

            # ---- transpose A locally ----
            at = []
            for j in range(DCH):
                tp = pst.tile([128, NM], f32, name="tp")
                nc.tensor.transpose(tp[:], a[:, j * 128:(j + 1) * 128],
                                    ident[0:NM, 0:NM])
                a1 = sb.tile([128, NM], f32r, name=f"at{j}")
                nc.vector.tensor_copy(a1[:], tp[:])
                at.append(a1)

            # ---- phase 2 (all local) ----
            # C^T[j_loc, i] = A_loc[j_loc] . E_all[i]
            c_ps = ps.tile([NM, N], f32)
            for j in range(DCH):
                st, sp = (j == 0), (j == DCH - 1)
                nc.tensor.matmul(c_ps[:], at[j][:], et[j][:], start=st, stop=sp)
            # raw C^T in SBUF (for the texts-direction transposes)
            csb = sb.tile([NM, N], f32)
            nc.scalar.copy(csb[:], c_ps[:])
            # G[i-tile, j_loc] = transpose of C^T columns, i tiled by 128
            gt = []
            for it in range(ITS):
                g1 = pst.tile([128, NM], f32, name="gt")
                nc.tensor.transpose(g1[:], csb[:, it * 128:(it + 1) * 128],
                                    ident[0:NM, 0:NM])
                gt.append(g1)

            out_sb = sb.tile([NM, 4], f32)

            # pts_loss (column softmax) fully on device from C^T rows
            mC = sb.tile([NM, 1], f32)
            nc.vector.reduce_max(mC[:], c_ps[:], axis=mybir.AxisListType.X)
            negsmC = sb.tile([NM, 1], f32)
            nc.vector.tensor_mul(negsmC[:], mC[:], sca[0:NM, :])
            nc.vector.tensor_scalar_mul(negsmC[:], negsmC[:], -1.0)
            exC = sb.tile([NM, N], f32)
            seC = sb.tile([NM, 1], f32)
            nc.scalar.activation(exC[:], c_ps[:], mybir.ActivationFunctionType.Exp,
                                 bias=negsmC[:], scale=sca[0:NM, :], accum_out=seC[:])
            lseC = sb.tile([NM, 1], f32)
            nc.scalar.activation(lseC[:], seC[:], mybir.ActivationFunctionType.Ln)
            tC = sb.tile([NM, 1], f32)
            nc.vector.tensor_sub(tC[:], mC[:], diag[:])
            nc.vector.tensor_mul(tC[:], tC[:], sca[0:NM, :])
            nc.vector.tensor_add(out_sb[:, 1:2], lseC[:], tC[:])

            # texts-direction row partials from G
            m4 = sb.tile([128, ITS], f32)
            s4 = sb.tile([128, ITS], f32)
            negm4 = sb.tile([128, ITS], f32)
            for it in range(ITS):
                g_sl = gt[it][:]
                nc.vector.reduce_max(m4[:, it:it + 1], g_sl, axis=mybir.AxisListType.X)
                nc.vector.tensor_mul(negm4[:, it:it + 1], m4[:, it:it + 1], sca[:])
                nc.vector.tensor_scalar_mul(negm4[:, it:it + 1], negm4[:, it:it + 1],
                                            -1.0)
                ex4 = sb.tile([128, NM], f32, name="ex4")
                nc.scalar.activation(ex4[:], g_sl, mybir.ActivationFunctionType.Exp,
                                     bias=negm4[:, it:it + 1], scale=sca[:],
                                     accum_out=s4[:, it:it + 1])

            # ---- all-gather the [512, 2] row partials (4 KB per rank) ----
            cc_in = dram.tile([N, 2], f32)
            cc_out = dram.tile([NCORES * N, 2], f32)
            for it in range(ITS):
                nc.sync.dma_start(cc_in[it * 128:(it + 1) * 128, 0:1], m4[:, it:it + 1])
                nc.sync.dma_start(cc_in[it * 128:(it + 1) * 128, 1:2], s4[:, it:it + 1])
            nc.gpsimd.collective_compute(
                "AllGather",
                mybir.AluOpType.bypass,
                replica_groups=[list(range(NCORES))],
                ins=[cc_in[:].opt()],
                outs=[cc_out[:].opt()],
            )
            gm = sb.tile([128, ITS, NCORES], f32)
            gs = sb.tile([128, ITS, NCORES], f32)
            for r in range(NCORES):
                src = cc_out[r * N:(r + 1) * N, :].rearrange("(i p) c -> p i c", p=128)
                nc.sync.dma_start(gm[:, :, r], src[:, :, 0])
                nc.sync.dma_start(gs[:, :, r], src[:, :, 1])

            # combine: M = max_r m_r ; T = sum_r s_r * exp(s*(m_r - M))
            Mx = sb.tile([128, ITS], f32)
            nc.vector.tensor_copy(Mx[:], gm[:, :, 0])
            for r in range(1, NCORES):
                nc.vector.tensor_max(Mx[:], Mx[:], gm[:, :, r])
            T = sb.tile([128, ITS], f32)
            df = sb.tile([128, ITS], f32)
            er = sb.tile([128, ITS], f32)
            tr = sb.tile([128, ITS], f32)
            for r in range(NCORES):
                nc.vector.tensor_sub(df[:], gm[:, :, r], Mx[:])
                nc.scalar.activation(er[:], df[:], mybir.ActivationFunctionType.Exp,
                                     bias=0.0, scale=sca[:])
                if r == 0:
                    nc.vector.tensor_mul(T[:], gs[:, :, r], er[:])
                else:
                    nc.vector.tensor_mul(tr[:], gs[:, :, r], er[:])
                    nc.vector.tensor_add(T[:], T[:], tr[:])
            lse4 = sb.tile([128, ITS], f32)
            nc.scalar.activation(lse4[:], T[:], mybir.ActivationFunctionType.Ln)
            sM = sb.tile([128, ITS], f32)
            for it in range(ITS):
                nc.vector.tensor_mul(sM[:, it:it + 1], Mx[:, it:it + 1], sca[:])
            nc.vector.tensor_add(lse4[:], lse4[:], sM[:])
            nc.sync.dma_start(lse_d[:], lse4[:])
            nc.sync.dma_start(dbg_d[:, 0:ITS], m4[:])
            nc.sync.dma_start(dbg_d[:, ITS:2 * ITS], s4[:])
            nc.sync.dma_start(dbg_d[:, 2 * ITS:2 * ITS + ITS * NCORES], gm[:])
            nc.sync.dma_start(dbg_d[:, 2 * ITS + ITS * NCORES:], gs[:])

            # out: col0 = s*diag, col1 = pts_loss, col2 = npts, col3 = diag
            nc.vector.tensor_mul(out_sb[:, 0:1], diag[:], sca[0:NM, :])
            nc.vector.tensor_copy(out_sb[:, 2:3], npts[:])
            nc.vector.tensor_copy(out_sb[:, 3:4], diag[:])
            nc.sync.dma_start(out_d[:], out_sb[:])
    nc.compile()
    return nc


def _prep_inputs(net_out, mask_embs, mask_pts, logit_scale):
    net_out = np.asarray(net_out, dtype=np.float32)
    mask_embs = np.ascontiguousarray(np.asarray(mask_embs, dtype=np.float32))
    mask_pts = np.asarray(mask_pts, dtype=np.float32)
    s = float(np.exp(np.float64(np.asarray(logit_scale).reshape(-1)[0])))

    # pts with two ones-columns, bf16
    pts_all = np.ones((BS * P, DP), dtype=ml_dtypes.bfloat16)
    pts_all[:, 0:D] = net_out.astype(ml_dtypes.bfloat16)

    # block-diagonal transposed masks: h[c, p, b*KPO+k, b*M+m] = mask[2c+b, m, k*128+p]
    v = mask_pts.reshape(NCORES, OBJ, M, KPO, 128).transpose(0, 1, 4, 3, 2)
    h = np.zeros((NCORES, 128, NCH, NM), dtype=ml_dtypes.bfloat16)
    for b in range(OBJ):
        h[:, :, b * KPO:(b + 1) * KPO, b * M:(b + 1) * M] = v[:, b]
    embsT = np.ascontiguousarray(mask_embs.T)
    sca = np.full((128, 1), s, dtype=np.float32)

    in_maps = []
    for c in range(NCORES):
        in_maps.append({
            "maskT": np.ascontiguousarray(h[c].reshape(128, NCH * NM)),
            "pts": pts_all[c * OBJ * P:(c + 1) * OBJ * P, :],
            "embsT": embsT,
            "eloc": np.ascontiguousarray(mask_embs[c * NM:(c + 1) * NM, :]),
            "sca": sca,
        })
    return in_maps, s


def _nonzero_mean(x):
    nz = x > 0
    cnt = int(nz.sum())
    if cnt == 0:
        return np.float32(0.0)
    return np.where(nz, x, 0.0).sum(dtype=np.float64) / cnt


def _combine(results, s):
    outs = [np.asarray(results[c]["out"]) for c in range(NCORES)]
    sdiag = np.concatenate([o[:, 0] for o in outs])
    ptsl = np.concatenate([o[:, 1] for o in outs])
    npts = np.concatenate([o[:, 2] for o in outs])
    # lse[p, it] = LSE[it*128 + p]; identical on every core -> take core 0
    lse = np.asarray(results[0]["lse"]).T.reshape(N)
    texts = lse - sdiag
    valid = npts > 0
    texts = np.where(valid, texts, 0.0)
    ptsl = np.where(valid, ptsl, 0.0)
    return np.asarray(
        (_nonzero_mean(texts) + _nonzero_mean(ptsl)) / 2.0, dtype=np.float32)


def _run(trace=False, **inputs):
    global _nc_cache
    if _nc_cache is None:
        _nc_cache = _build()
    in_maps, s = _prep_inputs(
        inputs["net_out"], inputs["mask_embs"], inputs["mask_pts"],
        inputs["logit_scale"])
    res = run_bass_kernel_spmd(
        _nc_cache, in_maps, core_ids=list(range(NCORES)), trace=trace)
    return _combine(res.results, s), res


def kernel(**inputs) -> np.ndarray:
    out, _ = _run(trace=False, **inputs)
    return out


# revision 10
# speedup vs baseline: 2.8975x; 2.0737x over previous
"""PointContrastive loss on 8 Trainium2 NeuronCores.

Sharding: data-parallel over objects (2 objects/core). Each core computes its
64 masks' summed/avg features via bf16 segment-matmuls (mask values and the
appended ones-columns are exact in bf16; accumulation is fp32 in PSUM), then
its local logit slab C^T = A_loc @ E_all^T [64 local j, 512 global i] from
purely local data. From that slab it derives:
  - pts_loss ingredients (row max + sum-exp over all 512 columns)
  - texts-direction partials (per global row i: max and shifted sum-exp over
    the local 64 columns), via 4 PE transposes of the slab.
Per-core device output is ~4.5 KB; the host does the 8-way log-sum-exp
combine and the two nonzero-means (pure O(N) postprocessing on 8 KB).
There is no cross-core communication on the device, so cores run fully
independently.

Device input layouts are chosen so every DMA is one long contiguous
descriptor per partition:
 - maskT: [128, 64*64] bf16  block-diagonal transposed masks (chunk-major)
 - pts:   [128, 64*770] bf16 partition-major point features: element
          [p, c*770 + m] = net_out[core_slab + c*128 + p, m], with columns
          768/769 of each chunk = 1.0 (npts falls out of the same matmul)
 - embsT: [768, 512] f32r    mask_embs transposed (same on all cores)
 - eloc:  [64, 768] f32      this core's row slice of mask_embs
"""

import numpy as np
import ml_dtypes
import concourse.bass as bass
from concourse import bacc
import concourse.mybir as mybir
import concourse.tile as tile
from concourse.bass_utils import run_bass_kernel_spmd
from concourse.masks import make_identity

BS, P, M, D = 16, 4096, 32, 768
NCORES = 8
OBJ = BS // NCORES          # objects per core = 2
NM = OBJ * M                # local masks = 64
N = BS * M                  # global masks = 512
KPO = P // 128              # k-chunks per object = 32
NCH = OBJ * KPO             # k-chunks per core = 64
DCH = D // 128              # feature chunks = 6
DP = D + 2                  # pts row width incl. ones columns = 770
ITS = N // 128              # row tiles of the global mask axis = 4
GRP = 4                     # k-chunks per pts DMA
NGRP = NCH // GRP           # pts DMAs = 16

f32 = mybir.dt.float32
f32r = mybir.dt.float32r
bf16 = mybir.dt.bfloat16

_nc_cache = None


def _build():
    nc = bacc.Bacc("TRN2", target_bir_lowering=False, debug=False, num_devices=NCORES)
    maskT_d = nc.dram_tensor("maskT", [128, NCH * NM], bf16, kind="ExternalInput").ap()
    pts_d = nc.dram_tensor("pts", [128, NCH * DP], bf16, kind="ExternalInput").ap()
    embsT_d = nc.dram_tensor("embsT", [D, N], f32r, kind="ExternalInput").ap()
    eloc_d = nc.dram_tensor("eloc", [NM, D], f32, kind="ExternalInput").ap()
    sca_d = nc.dram_tensor("sca", [128, 1], f32, kind="ExternalInput").ap()
    out_d = nc.dram_tensor("out", [NM, 4], f32, kind="ExternalOutput").ap()
    st_d = nc.dram_tensor("stats", [128, 2 * ITS], f32, kind="ExternalOutput").ap()

    with tile.TileContext(nc) as tc:
        with (
            tc.tile_pool(name="sb", bufs=1) as sb,
            tc.tile_pool(name="pts_pool", bufs=4) as pts_pool,
            tc.tile_pool(name="ps", bufs=1, space="PSUM") as ps,
            tc.tile_pool(name="pst", bufs=2, space="PSUM") as pst,
        ):
            # ---- persistent loads (ACT HWDGE queue; pts stream owns Sync) ----
            ident = sb.tile([128, 128], f32)
            make_identity(nc, ident[:])
            sca = sb.tile([128, 1], f32)
            nc.scalar.dma_start(sca[:], sca_d[:])
            mt = sb.tile([128, NCH * NM], bf16)
            nc.scalar.dma_start(mt[:], maskT_d[:])
            el = sb.tile([NM, D], f32)
            nc.scalar.dma_start(el[:], eloc_d[:])
            et = []
            for j in range(DCH):
                e1 = sb.tile([128, N], f32r, name=f"et{j}")
                nc.scalar.dma_start(e1[:], embsT_d[j * 128:(j + 1) * 128, :])
                et.append(e1)

            # ---- phase 1: segment sums over 64 k-chunks, 4 chunks per DMA ----
            s_ps = ps.tile([NM, DP], f32)
            for g in range(NGRP):
                pt = pts_pool.tile([128, GRP * DP], bf16, name="pt")
                nc.sync.dma_start(pt[:], pts_d[:, g * GRP * DP:(g + 1) * GRP * DP])
                for q in range(GRP):
                    c = GRP * g + q
                    lhs = mt[:, c * NM:(c + 1) * NM]
                    st, sp = (c == 0), (c == NCH - 1)
                    nc.tensor.matmul(s_ps[:, 0:512], lhs,
                                     pt[:, q * DP:q * DP + 512], start=st, stop=sp)
                    nc.tensor.matmul(s_ps[:, 512:DP], lhs,
                                     pt[:, q * DP + 512:(q + 1) * DP], start=st, stop=sp)

            npts = sb.tile([NM, 1], f32)
            nc.vector.tensor_copy(npts[:], s_ps[:, D:D + 1])
            t1 = sb.tile([NM, 1], f32)
            nc.vector.tensor_scalar_add(t1[:], npts[:], 1e-12)
            inv = sb.tile([NM, 1], f32)
            nc.vector.reciprocal(inv[:], t1[:])
            a = sb.tile([NM, D], f32)
            nc.vector.tensor_scalar_mul(a[:], s_ps[:, 0:D], inv[:])

            # diag[i] = eloc[i] . a[i]
            dtmp = sb.tile([NM, D], f32)
            nc.vector.tensor_mul(dtmp[:], a[:], el[:])
            diag = sb.tile([NM, 1], f32)
            nc.vector.reduce_sum(diag[:], dtmp[:], axis=mybir.AxisListType.X)

            # ---- transpose A locally (A^T chunks feed the C matmuls) ----
            at = []
            for j in range(DCH):
                tp = pst.tile([128, NM], f32, name="tp")
                nc.tensor.transpose(tp[:], a[:, j * 128:(j + 1) * 128],
                                    ident[0:NM, 0:NM])
                a1 = sb.tile([128, NM], f32r, name=f"at{j}")
                nc.vector.tensor_copy(a1[:], tp[:])
                at.append(a1)

            # ---- phase 2 (all local): C^T[j_loc, i] = A_loc[j_loc] . E_all[i]
            c_ps = ps.tile([NM, N], f32)
            for j in range(DCH):
                nc.tensor.matmul(c_ps[:], at[j][:], et[j][:],
                                 start=(j == 0), stop=(j == DCH - 1))

            out_sb = sb.tile([NM, 4], f32)
            # pts_loss ingredients: row max + sum-exp of s*(C - mC)
            mC = sb.tile([NM, 1], f32)
            nc.vector.reduce_max(mC[:], c_ps[:], axis=mybir.AxisListType.X)
            negsmC = sb.tile([NM, 1], f32)
            nc.vector.tensor_mul(negsmC[:], mC[:], sca[0:NM, :])
            nc.vector.tensor_scalar_mul(negsmC[:], negsmC[:], -1.0)
            exC = sb.tile([NM, N], f32)
            seC = sb.tile([NM, 1], f32)
            nc.scalar.activation(exC[:], c_ps[:], mybir.ActivationFunctionType.Exp,
                                 bias=negsmC[:], scale=sca[0:NM, :], accum_out=seC[:])
            # raw C^T in SBUF for the texts-direction transposes
            csb = sb.tile([NM, N], f32)
            nc.vector.tensor_copy(csb[:], c_ps[:])

            st8 = sb.tile([128, 2 * ITS], f32)
            negm = sb.tile([128, ITS], f32)
            for it in range(ITS):
                g1 = pst.tile([128, NM], f32, name="gt")
                nc.tensor.transpose(g1[:], csb[:, it * 128:(it + 1) * 128],
                                    ident[0:NM, 0:NM])
                nc.vector.reduce_max(st8[:, it:it + 1], g1[:],
                                     axis=mybir.AxisListType.X)
                nc.vector.tensor_mul(negm[:, it:it + 1], st8[:, it:it + 1], sca[:])
                nc.vector.tensor_scalar_mul(negm[:, it:it + 1], negm[:, it:it + 1],
                                            -1.0)
                ex4 = sb.tile([128, NM], f32, name="ex4")
                nc.scalar.activation(ex4[:], g1[:], mybir.ActivationFunctionType.Exp,
                                     bias=negm[:, it:it + 1], scale=sca[:],
                                     accum_out=st8[:, ITS + it:ITS + it + 1])
            nc.sync.dma_start(st_d[:], st8[:])

            # out: col0 = diag, col1 = mC, col2 = seC, col3 = npts
            nc.vector.tensor_copy(out_sb[:, 0:1], diag[:])
            nc.vector.tensor_copy(out_sb[:, 1:2], mC[:])
            nc.vector.tensor_copy(out_sb[:, 2:3], seC[:])
            nc.vector.tensor_copy(out_sb[:, 3:4], npts[:])
            nc.sync.dma_start(out_d[:], out_sb[:])
    nc.compile()
    return nc


def _prep_inputs(net_out, mask_embs, mask_pts, logit_scale):
    net_out = np.asarray(net_out, dtype=np.float32)
    mask_embs = np.ascontiguousarray(np.asarray(mask_embs, dtype=np.float32))
    mask_pts = np.asarray(mask_pts, dtype=np.float32)
    s = float(np.exp(np.float64(np.asarray(logit_scale).reshape(-1)[0])))

    # partition-major pts with ones-columns: [core, p, c, m]
    pts_all = np.ones((BS * P, DP), dtype=ml_dtypes.bfloat16)
    pts_all[:, 0:D] = net_out.astype(ml_dtypes.bfloat16)
    # [core, c, p, m] -> [core, p, c, m]
    pts_pm = pts_all.reshape(NCORES, NCH, 128, DP).transpose(0, 2, 1, 3)

    # block-diagonal transposed masks: h[c, p, b*KPO+k, b*M+m] = mask[2c+b, m, k*128+p]
    v = mask_pts.reshape(NCORES, OBJ, M, KPO, 128).transpose(0, 1, 4, 3, 2)
    h = np.zeros((NCORES, 128, NCH, NM), dtype=ml_dtypes.bfloat16)
    for b in range(OBJ):
        h[:, :, b * KPO:(b + 1) * KPO, b * M:(b + 1) * M] = v[:, b]
    embsT = np.ascontiguousarray(mask_embs.T)
    sca = np.full((128, 1), s, dtype=np.float32)

    in_maps = []
    for c in range(NCORES):
        in_maps.append({
            "maskT": np.ascontiguousarray(h[c].reshape(128, NCH * NM)),
            "pts": np.ascontiguousarray(pts_pm[c].reshape(128, NCH * DP)),
            "embsT": embsT,
            "eloc": np.ascontiguousarray(mask_embs[c * NM:(c + 1) * NM, :]),
            "sca": sca,
        })
    return in_maps, s


def _nonzero_mean(x):
    nz = x > 0
    cnt = int(nz.sum())
    if cnt == 0:
        return np.float32(0.0)
    return np.where(nz, x, 0.0).sum(dtype=np.float64) / cnt


def _combine(results, s):
    outs = [np.asarray(results[c]["out"]) for c in range(NCORES)]
    diag = np.concatenate([o[:, 0] for o in outs]).astype(np.float64)   # [512]
    mC = np.concatenate([o[:, 1] for o in outs]).astype(np.float64)     # raw row max
    seC = np.concatenate([o[:, 2] for o in outs]).astype(np.float64)    # sum exp(s*(C-mC))
    npts = np.concatenate([o[:, 3] for o in outs])
    # texts-direction partials: stats[p, it] (raw max), stats[p, ITS+it] (sumexp)
    m_r = np.stack([np.asarray(results[c]["stats"])[:, 0:ITS].T.reshape(N)
                    for c in range(NCORES)]).astype(np.float64)         # [8, 512]
    s_r = np.stack([np.asarray(results[c]["stats"])[:, ITS:2 * ITS].T.reshape(N)
                    for c in range(NCORES)]).astype(np.float64)
    Mx = m_r.max(axis=0)
    T = (s_r * np.exp(s * (m_r - Mx))).sum(axis=0)
    lse_rows = np.log(T) + s * Mx
    texts = lse_rows - s * diag
    ptsl = np.log(seC) + s * mC - s * diag
    valid = npts > 0
    texts = np.where(valid, texts, 0.0)
    ptsl = np.where(valid, ptsl, 0.0)
    return np.asarray(
        (_nonzero_mean(texts) + _nonzero_mean(ptsl)) / 2.0, dtype=np.float32)


def _run(trace=False, **inputs):
    global _nc_cache
    if _nc_cache is None:
        _nc_cache = _build()
    in_maps, s = _prep_inputs(
        inputs["net_out"], inputs["mask_embs"], inputs["mask_pts"],
        inputs["logit_scale"])
    res = run_bass_kernel_spmd(
        _nc_cache, in_maps, core_ids=list(range(NCORES)), trace=trace)
    return _combine(res.results, s), res


def kernel(**inputs) -> np.ndarray:
    out, _ = _run(trace=False, **inputs)
    return out
